# revision 6
# baseline (speedup 1.0000x reference)
"""Two-layer GCN feature extractor on 8 Trainium2 NeuronCores.

Strategy (self-contained; all shapes hardcoded for the target problem):
  * Nodes are sharded across 8 cores (6250 rows each).  Each core:
      1. transforms its own rows: h = dinv * (x_own @ W)         (PE)
      2. AllGather -> replicated h table [N, F] in DRAM          (collective)
      3. gathers per-edge source rows with dma_gather            (SWDGE DMA)
      4. segment-sums messages into its destination rows via
         one-hot selection matmuls accumulated in PSUM           (PE + DVE)
      5. y = relu(dinv * psum + b)  (layer 1), then repeat for layer 2.
  * Graph preprocessing (edge partitioning by destination, sorting,
    degree counting, int16 index stream layout) happens on host with
    numpy; all floating-point math runs on device.
  * dma_gather needs int16 indices, so the node table is addressed as
    two halves: src < 32768 ("lo", table base row 0) and src >= 32768
    ("hi", table base row 32768).
"""

import math
import os
from contextlib import ExitStack

import numpy as np

os.environ.setdefault("MYCRO_LOCAL_CACHE", "1")

# ----------------------------------------------------------------------------
# configuration
# ----------------------------------------------------------------------------


def make_cfg(N=50000, F=128, ncores=8, split=32768, gchunks=8, table_bf16=False):
    assert N % ncores == 0
    rows = N // ncores
    nblk = math.ceil(rows / 128)
    return dict(
        N=N,
        F=F,
        ncores=ncores,
        split=split,
        rows=rows,
        nblk=nblk,
        last_rows=rows - (nblk - 1) * 128,
        gchunks=gchunks,
        table_bf16=table_bf16,
    )


FULL_CFG = make_cfg()


# ----------------------------------------------------------------------------
# host-side graph preprocessing
# ----------------------------------------------------------------------------


def preprocess(edge_index, cfg):
    """Partition edges by destination core, sort by (dst block, src half),
    pad each (block, half) run to a multiple of 128, and lay out index /
    dst-local streams in the formats dma_gather and the kernel expect.

    Returns (sched, per_core, deg) where sched = (c_lo, c_hi) chunk counts
    per block (uniform across cores).
    """
    N, ncores, rows, nblk, split = (
        cfg["N"],
        cfg["ncores"],
        cfg["rows"],
        cfg["nblk"],
        cfg["split"],
    )

    src = np.asarray(edge_index[0], dtype=np.int64)
    dst = np.asarray(edge_index[1], dtype=np.int64)
    loops = np.arange(N, dtype=np.int64)
    src = np.concatenate([src, loops])
    dst = np.concatenate([dst, loops])

    deg = np.bincount(dst, minlength=N).astype(np.float32)  # >= 1 (self loops)

    core_of = dst // rows
    per_core_raw = []
    counts = np.zeros((ncores, nblk, 2), dtype=np.int64)
    for k in range(ncores):
        m = core_of == k
        s_k = src[m]
        d_k = dst[m] - k * rows
        blk = d_k >> 7
        dl = (d_k & 127).astype(np.float32)
        half = (s_k >= split).astype(np.int64)
        order = np.lexsort((s_k, half, blk))
        s_k, dl, blk, half = s_k[order], dl[order], blk[order], half[order]
        c = np.bincount(blk * 2 + half, minlength=nblk * 2).reshape(nblk, 2)
        counts[k] = c
        per_core_raw.append((s_k, dl, c))

    cdiv = lambda a, b: -(-a // b)
    c_lo = [int(max(cdiv(counts[k, b, 0], 128) for k in range(ncores))) for b in range(nblk)]
    c_hi = [int(max(cdiv(counts[k, b, 1], 128) for k in range(ncores))) for b in range(nblk)]
    S_lo = sum(c_lo) * 128
    S_hi = sum(c_hi) * 128

    per_core = []
    for k in range(ncores):
        s_k, dl_k, c = per_core_raw[k]
        # per-(block, half) start offsets into the sorted arrays
        starts = np.concatenate([[0], np.cumsum(c.reshape(-1))])
        idx_lo = np.zeros(S_lo, np.int16)
        dst_lo = np.full(S_lo, -1.0, np.float32)
        idx_hi = np.zeros(S_hi, np.int16)
        dst_hi = np.full(S_hi, -1.0, np.float32)
        plo = phi = 0
        for b in range(nblk):
            n0 = int(c[b, 0])
            o0 = int(starts[b * 2])
            idx_lo[plo : plo + n0] = s_k[o0 : o0 + n0].astype(np.int16)
            dst_lo[plo : plo + n0] = dl_k[o0 : o0 + n0]
            plo += c_lo[b] * 128
            n1 = int(c[b, 1])
            o1 = int(starts[b * 2 + 1])
            idx_hi[phi : phi + n1] = (s_k[o1 : o1 + n1] - split).astype(np.int16)
            dst_hi[phi : phi + n1] = dl_k[o1 : o1 + n1]
            phi += c_hi[b] * 128

        def arrange_idx(a):  # logical i -> sbuf[i % 16, i // 16], tiled to 128 parts
            if a.size == 0:
                return np.zeros((128, 0), np.int16)
            return np.tile(np.ascontiguousarray(a.reshape(-1, 16).T), (8, 1))

        def arrange_dl(a):  # logical i -> sbuf[i % 128, i // 128]
            if a.size == 0:
                return np.zeros((128, 0), np.float32)
            return np.ascontiguousarray(a.reshape(-1, 128).T)

        degk = np.ones(nblk * 128, np.float32)
        degk[:rows] = deg[k * rows : (k + 1) * rows]

        per_core.append(
            dict(
                idx_lo=arrange_idx(idx_lo),
                idx_hi=arrange_idx(idx_hi),
                dl_lo=arrange_dl(dst_lo),
                dl_hi=arrange_dl(dst_hi),
                deg=np.ascontiguousarray(degk.reshape(nblk, 128).T),
            )
        )

    return (tuple(c_lo), tuple(c_hi)), per_core, deg


# ----------------------------------------------------------------------------
# bass program
# ----------------------------------------------------------------------------

_PROGRAM_CACHE = {}


def build_program(cfg, sched):
    import concourse.bacc as bacc
    import concourse.bass as bass
    import concourse.mybir as mybir
    import concourse.tile as tile
    from concourse.masks import make_identity

    c_lo, c_hi = sched
    N, F, ncores, split = cfg["N"], cfg["F"], cfg["ncores"], cfg["split"]
    rows, nblk, last_rows = cfg["rows"], cfg["nblk"], cfg["last_rows"]
    gchunks = cfg["gchunks"]
    S_lo, S_hi = sum(c_lo) * 128, sum(c_hi) * 128

    f32 = mybir.dt.float32
    i16 = mybir.dt.int16
    TD = mybir.dt.bfloat16 if cfg["table_bf16"] else f32
    mul = mybir.AluOpType.mult
    add = mybir.AluOpType.add
    eq = mybir.AluOpType.is_equal
    mx = mybir.AluOpType.max

    nc = bacc.Bacc(
        "TRN2",
        target_bir_lowering=False,
        debug=False,
        enable_asserts=False,
        num_devices=ncores,
    )

    x_own = nc.dram_tensor("x_own", [nblk * 128, F], f32, kind="ExternalInput")
    W1d = nc.dram_tensor("W1", [F, F], f32, kind="ExternalInput")
    W2d = nc.dram_tensor("W2", [F, F], f32, kind="ExternalInput")
    b1d = nc.dram_tensor("b1_bc", [128, F], f32, kind="ExternalInput")
    b2d = nc.dram_tensor("b2_bc", [128, F], f32, kind="ExternalInput")
    degd = nc.dram_tensor("deg", [128, nblk], f32, kind="ExternalInput")
    iotad = nc.dram_tensor("iota", [128, 128], f32, kind="ExternalInput")
    ixlod = ixhid = dllod = dlhid = None
    if S_lo:
        ixlod = nc.dram_tensor("idx_lo", [128, S_lo // 16], i16, kind="ExternalInput")
        dllod = nc.dram_tensor("dl_lo", [128, S_lo // 128], f32, kind="ExternalInput")
    if S_hi:
        ixhid = nc.dram_tensor("idx_hi", [128, S_hi // 16], i16, kind="ExternalInput")
        dlhid = nc.dram_tensor("dl_hi", [128, S_hi // 128], f32, kind="ExternalInput")
    yout = nc.dram_tensor("y_out", [rows, F], f32, kind="ExternalOutput")

    h_stage = [nc.dram_tensor(f"h_stage{i}", [rows, F], TD) for i in (1, 2)]
    h_full = [
        nc.dram_tensor(f"h_full{i}", [N, F], TD, addr_space="Shared") for i in (1, 2)
    ]

    with tile.TileContext(nc) as tc, ExitStack() as ctx:
        const = ctx.enter_context(tc.tile_pool(name="const", bufs=1))
        xf = ctx.enter_context(tc.tile_pool(name="xf", bufs=3))
        xfp = ctx.enter_context(tc.tile_pool(name="xfp", bufs=2, space="PSUM"))
        gps = ctx.enter_context(tc.tile_pool(name="gps", bufs=2, space="PSUM"))
        glo = ctx.enter_context(tc.tile_pool(name="glo", bufs=2))
        ghi = ctx.enter_context(tc.tile_pool(name="ghi", bufs=2))
        selp = ctx.enter_context(tc.tile_pool(name="selp", bufs=4))
        outp = ctx.enter_context(tc.tile_pool(name="outp", bufs=3))

        # ---- constants ---------------------------------------------------
        def load_const(dram, shape, dtype):
            t = const.tile(shape, dtype, tag=f"c_{dram.name}")
            nc.sync.dma_start(t[:], dram[:])
            return t

        W1s = load_const(W1d, [F, F], f32)
        W2s = load_const(W2d, [F, F], f32)
        b1s = load_const(b1d, [128, F], f32)
        b2s = load_const(b2d, [128, F], f32)
        iota = load_const(iotad, [128, 128], f32)
        degs = load_const(degd, [128, nblk], f32)
        ixlo = load_const(ixlod, [128, S_lo // 16], i16) if S_lo else None
        dllo = load_const(dllod, [128, S_lo // 128], f32) if S_lo else None
        ixhi = load_const(ixhid, [128, S_hi // 16], i16) if S_hi else None
        dlhi = load_const(dlhid, [128, S_hi // 128], f32) if S_hi else None

        dinv = const.tile([128, nblk], f32)
        nc.vector.reciprocal(dinv[:], degs[:])
        nc.scalar.activation(dinv[:], dinv[:], mybir.ActivationFunctionType.Sqrt)

        ident = const.tile([128, 128], f32)
        make_identity(nc, ident[:])

        y1 = const.tile([128, nblk * 128], f32)  # layer-1 output, feature cols

        # ---- transform: h_stage = dinv * (rows @ W) ----------------------
        def transform(get_tile, W_s, stage):
            for t in range(nblk):
                xt = get_tile(t)
                pT = xfp.tile([128, 128], f32)
                nc.tensor.transpose(pT[:], xt[:], ident[:])
                xT = xf.tile([128, 128], f32)
                nc.vector.tensor_copy(xT[:], pT[:])
                ph = xfp.tile([128, F], f32)
                nc.tensor.matmul(ph[:], lhsT=xT[:], rhs=W_s[:], start=True, stop=True)
                hs = xf.tile([128, F], TD)
                nc.vector.tensor_scalar(
                    out=hs[:], in0=ph[:], scalar1=dinv[:, t : t + 1], scalar2=None,
                    op0=mul,
                )
                r = 128 if t < nblk - 1 else last_rows
                nc.sync.dma_start(stage[t * 128 : t * 128 + r, :], hs[:r, :])

        def x_tile(t):
            xt = xf.tile([128, F], f32)
            nc.sync.dma_start(xt[:], x_own[t * 128 : (t + 1) * 128, :])
            return xt

        # ---- gather + segment-sum reduce ---------------------------------
        class Stream:
            def __init__(self, idx, dl, view, total_chunks, pool):
                self.idx, self.dl, self.view, self.pool = idx, dl, view, pool
                self.total = total_chunks
                self.pos = 0
                self.slab = None
                self.base = 0
                self.n = 0

            def chunk(self):
                if self.slab is None or self.pos >= self.base + self.n:
                    self.base = self.pos
                    self.n = min(gchunks, self.total - self.pos)
                    nidx = self.n * 128
                    self.slab = self.pool.tile([128, self.n, F], TD, tag="slab")
                    nc.gpsimd.dma_gather(
                        self.slab[:],
                        self.view,
                        self.idx[:, self.pos * 8 : self.pos * 8 + nidx // 16],
                        nidx,
                        nidx,
                        F,
                    )
                col = self.pos - self.base
                dl_col = self.dl[:, self.pos : self.pos + 1]
                self.pos += 1
                return self.slab, col, dl_col

        def gather_reduce(hf, finalize):
            streams = []
            if S_lo:
                streams.append(
                    (c_lo, Stream(ixlo, dllo, hf[:], S_lo // 128, glo))
                )
            if S_hi:
                streams.append(
                    (c_hi, Stream(ixhi, dlhi, hf[split:, :], S_hi // 128, ghi))
                )
            for b in range(nblk):
                py = gps.tile([128, F], f32)
                nch = sum(c[b] for c, _ in streams)
                assert nch > 0
                i = 0
                for c, st in streams:
                    for _ in range(c[b]):
                        slab, col, dl_col = st.chunk()
                        sel = selp.tile([128, 128], TD)
                        nc.vector.tensor_scalar(
                            out=sel[:], in0=iota[:], scalar1=dl_col, scalar2=None,
                            op0=eq,
                        )
                        nc.tensor.matmul(
                            py[:],
                            lhsT=sel[:],
                            rhs=slab[:, col, :],
                            start=(i == 0),
                            stop=(i == nch - 1),
                        )
                        i += 1
                finalize(b, py)

        # ---- layer 1 -----------------------------------------------------
        transform(x_tile, W1s, h_stage[0])
        nc.gpsimd.collective_compute(
            "AllGather",
            mybir.AluOpType.bypass,
            replica_groups=[list(range(ncores))],
            ins=[h_stage[0][:]],
            outs=[h_full[0][:]],
        )

        def fin1(b, py):
            ys = y1[:, b * 128 : (b + 1) * 128]
            nc.vector.scalar_tensor_tensor(
                out=ys, in0=py[:], scalar=dinv[:, b : b + 1], in1=b1s[:],
                op0=mul, op1=add,
            )
            nc.vector.tensor_scalar(
                out=ys, in0=ys, scalar1=0.0, scalar2=None, op0=mx
            )

        gather_reduce(h_full[0], fin1)

        # ---- layer 2 -----------------------------------------------------
        transform(lambda t: y1[:, t * 128 : (t + 1) * 128], W2s, h_stage[1])
        nc.gpsimd.collective_compute(
            "AllGather",
            mybir.AluOpType.bypass,
            replica_groups=[list(range(ncores))],
            ins=[h_stage[1][:]],
            outs=[h_full[1][:]],
        )

        def fin2(b, py):
            yt = outp.tile([128, F], f32)
            nc.vector.scalar_tensor_tensor(
                out=yt[:], in0=py[:], scalar=dinv[:, b : b + 1], in1=b2s[:],
                op0=mul, op1=add,
            )
            r = 128 if b < nblk - 1 else last_rows
            nc.sync.dma_start(yout[b * 128 : b * 128 + r, :], yt[:r, :])

        gather_reduce(h_full[1], fin2)

    nc.compile()
    return nc


def get_program(cfg, sched):
    key = (tuple(sorted(cfg.items())), sched)
    if key not in _PROGRAM_CACHE:
        _PROGRAM_CACHE[key] = build_program(cfg, sched)
    return _PROGRAM_CACHE[key]


# ----------------------------------------------------------------------------
# input marshalling + entry point
# ----------------------------------------------------------------------------


def make_in_maps(x, W1, b1, W2, b2, cfg, per_core):
    N, F, ncores, rows, nblk = (
        cfg["N"],
        cfg["F"],
        cfg["ncores"],
        cfg["rows"],
        cfg["nblk"],
    )
    x = np.asarray(x, np.float32)
    W1 = np.ascontiguousarray(np.asarray(W1, np.float32))
    W2 = np.ascontiguousarray(np.asarray(W2, np.float32))
    b1_bc = np.ascontiguousarray(np.broadcast_to(np.asarray(b1, np.float32), (128, F)))
    b2_bc = np.ascontiguousarray(np.broadcast_to(np.asarray(b2, np.float32), (128, F)))
    iota = np.ascontiguousarray(
        np.broadcast_to(np.arange(128, dtype=np.float32), (128, 128))
    )
    in_maps = []
    for k in range(ncores):
        xk = np.zeros((nblk * 128, F), np.float32)
        xk[:rows] = x[k * rows : (k + 1) * rows]
        pc = per_core[k]
        in_maps.append(
            dict(
                x_own=xk,
                W1=W1,
                W2=W2,
                b1_bc=b1_bc,
                b2_bc=b2_bc,
                deg=pc["deg"],
                iota=iota,
                idx_lo=pc["idx_lo"],
                idx_hi=pc["idx_hi"],
                dl_lo=pc["dl_lo"],
                dl_hi=pc["dl_hi"],
            )
        )
    return in_maps


def _ensure_ntff_hook():
    """Register the NTFF profiling hook (missing antenv.axon_hooks shim)."""
    try:
        from antenv.axon_hooks import get_axon_ntff_profile_hook  # noqa: F401

        return True
    except ImportError:
        pass
    try:
        import sys
        import types

        import antenv
        from trn_agent_boot.trn_boot import _ntff_profile_via_ctypes

        hook = _ntff_profile_via_ctypes("/opt/axon/libaxon_pjrt.so")
        if hook is None:
            return False
        mod = types.ModuleType("antenv.axon_hooks")
        mod._hook = hook
        mod.get_axon_ntff_profile_hook = lambda: mod._hook
        mod.set_axon_ntff_profile_hook = lambda h: setattr(mod, "_hook", h)
        sys.modules["antenv.axon_hooks"] = mod
        antenv.axon_hooks = mod
        # artifact upload needs cloud credentials; stub it out
        import concourse.bass_utils as bu

        bu.upload_artifacts = lambda tmpdir: f"local:{tmpdir}"
        return True
    except Exception:
        return False


def run(x, edge_index, W1, b1, W2, b2, cfg, trace=False):
    from concourse.bass_utils import run_bass_kernel_spmd

    if trace:
        trace = _ensure_ntff_hook()

    sched, per_core, _ = preprocess(edge_index, cfg)
    nc = get_program(cfg, sched)
    in_maps = make_in_maps(x, W1, b1, W2, b2, cfg, per_core)
    res = run_bass_kernel_spmd(
        nc, in_maps, list(range(cfg["ncores"])), trace=trace
    )
    out = np.concatenate(
        [res.results[k]["y_out"] for k in range(cfg["ncores"])], axis=0
    )
    return out.astype(np.float32), res


def kernel(x, edge_index, W1, b1, W2, b2):
    out, _ = run(x, edge_index, W1, b1, W2, b2, FULL_CFG)
    return out


# revision 14
# speedup vs baseline: 1.6521x; 1.6521x over previous
"""Two-layer GCN feature extractor on 8 Trainium2 NeuronCores.

Strategy (self-contained; all shapes hardcoded for the target problem):
  * Nodes are sharded across 8 cores (6250 rows each).  Each core:
      1. transforms its own rows: h = dinv * (x_own @ W)         (PE)
      2. AllGather -> replicated h table [N, F] in DRAM          (collective)
      3. gathers per-edge source rows with dma_gather            (SWDGE DMA)
      4. segment-sums messages into its destination rows via
         one-hot selection matmuls accumulated in PSUM           (PE + DVE)
      5. y = relu(dinv * psum + b)  (layer 1), then repeat for layer 2.
  * Graph preprocessing (edge partitioning by destination, sorting,
    degree counting, int16 index stream layout) happens on host with
    numpy; all floating-point math runs on device.
  * dma_gather needs int16 indices, so the node table is addressed as
    two halves: src < 32768 ("lo", table base row 0) and src >= 32768
    ("hi", table base row 32768).
"""

import math
import os
from contextlib import ExitStack

import numpy as np

os.environ.setdefault("MYCRO_LOCAL_CACHE", "1")

# ----------------------------------------------------------------------------
# configuration
# ----------------------------------------------------------------------------


def make_cfg(
    N=50000,
    F=128,
    ncores=8,
    split=32768,
    gchunks=8,
    table_bf16=False,
    nqueues=4,
    selb=4,
):
    assert N % ncores == 0
    rows = N // ncores
    nblk = math.ceil(rows / 128)
    return dict(
        N=N,
        F=F,
        ncores=ncores,
        split=split,
        rows=rows,
        nblk=nblk,
        last_rows=rows - (nblk - 1) * 128,
        gchunks=gchunks,
        table_bf16=table_bf16,
        nqueues=nqueues,
        selb=selb,
    )


FULL_CFG = make_cfg()


# ----------------------------------------------------------------------------
# host-side graph preprocessing
# ----------------------------------------------------------------------------


def preprocess(edge_index, cfg):
    """Partition edges by destination core, sort by (dst block, src half),
    pad each (block, half) run to a multiple of 128, and lay out index /
    dst-local streams in the formats dma_gather and the kernel expect.

    Returns (sched, per_core, deg) where sched = (c_lo, c_hi) chunk counts
    per block (uniform across cores).
    """
    N, ncores, rows, nblk, split = (
        cfg["N"],
        cfg["ncores"],
        cfg["rows"],
        cfg["nblk"],
        cfg["split"],
    )

    src = np.asarray(edge_index[0], dtype=np.int64)
    dst = np.asarray(edge_index[1], dtype=np.int64)
    loops = np.arange(N, dtype=np.int64)
    src = np.concatenate([src, loops])
    dst = np.concatenate([dst, loops])

    deg = np.bincount(dst, minlength=N).astype(np.float32)  # >= 1 (self loops)

    core_of = dst // rows
    per_core_raw = []
    counts = np.zeros((ncores, nblk, 2), dtype=np.int64)
    for k in range(ncores):
        m = core_of == k
        s_k = src[m]
        d_k = dst[m] - k * rows
        blk = d_k >> 7
        dl = (d_k & 127).astype(np.float32)
        half = (s_k >= split).astype(np.int64)
        order = np.lexsort((s_k, half, blk))
        s_k, dl, blk, half = s_k[order], dl[order], blk[order], half[order]
        c = np.bincount(blk * 2 + half, minlength=nblk * 2).reshape(nblk, 2)
        counts[k] = c
        per_core_raw.append((s_k, dl, c))

    cdiv = lambda a, b: -(-a // b)
    c_lo = [int(max(cdiv(counts[k, b, 0], 128) for k in range(ncores))) for b in range(nblk)]
    c_hi = [int(max(cdiv(counts[k, b, 1], 128) for k in range(ncores))) for b in range(nblk)]
    S_lo = sum(c_lo) * 128
    S_hi = sum(c_hi) * 128

    per_core = []
    for k in range(ncores):
        s_k, dl_k, c = per_core_raw[k]
        # per-(block, half) start offsets into the sorted arrays
        starts = np.concatenate([[0], np.cumsum(c.reshape(-1))])
        idx_lo = np.zeros(S_lo, np.int16)
        dst_lo = np.full(S_lo, -1.0, np.float32)
        idx_hi = np.zeros(S_hi, np.int16)
        dst_hi = np.full(S_hi, -1.0, np.float32)
        plo = phi = 0
        for b in range(nblk):
            n0 = int(c[b, 0])
            o0 = int(starts[b * 2])
            idx_lo[plo : plo + n0] = s_k[o0 : o0 + n0].astype(np.int16)
            dst_lo[plo : plo + n0] = dl_k[o0 : o0 + n0]
            plo += c_lo[b] * 128
            n1 = int(c[b, 1])
            o1 = int(starts[b * 2 + 1])
            idx_hi[phi : phi + n1] = (s_k[o1 : o1 + n1] - split).astype(np.int16)
            dst_hi[phi : phi + n1] = dl_k[o1 : o1 + n1]
            phi += c_hi[b] * 128

        def arrange_idx(a):  # logical i -> sbuf[i % 16, i // 16], tiled to 128 parts
            if a.size == 0:
                return np.zeros((128, 0), np.int16)
            return np.tile(np.ascontiguousarray(a.reshape(-1, 16).T), (8, 1))

        def arrange_dl(a):  # logical i -> sbuf[i % 128, i // 128]
            if a.size == 0:
                return np.zeros((128, 0), np.float32)
            return np.ascontiguousarray(a.reshape(-1, 128).T)

        degk = np.ones(nblk * 128, np.float32)
        degk[:rows] = deg[k * rows : (k + 1) * rows]

        per_core.append(
            dict(
                idx_lo=arrange_idx(idx_lo),
                idx_hi=arrange_idx(idx_hi),
                dl_lo=arrange_dl(dst_lo),
                dl_hi=arrange_dl(dst_hi),
                deg=np.ascontiguousarray(degk.reshape(nblk, 128).T),
            )
        )

    return (tuple(c_lo), tuple(c_hi)), per_core, deg


# ----------------------------------------------------------------------------
# bass program
# ----------------------------------------------------------------------------

_PROGRAM_CACHE = {}


def build_program(cfg, sched):
    import concourse.bacc as bacc
    import concourse.bass as bass
    import concourse.mybir as mybir
    import concourse.tile as tile
    from concourse.masks import make_identity

    c_lo, c_hi = sched
    N, F, ncores, split = cfg["N"], cfg["F"], cfg["ncores"], cfg["split"]
    rows, nblk, last_rows = cfg["rows"], cfg["nblk"], cfg["last_rows"]
    gchunks = cfg["gchunks"]
    S_lo, S_hi = sum(c_lo) * 128, sum(c_hi) * 128

    f32 = mybir.dt.float32
    i16 = mybir.dt.int16
    TD = mybir.dt.bfloat16 if cfg["table_bf16"] else f32
    nq = cfg.get("nqueues", 4)
    selb = cfg.get("selb", 4)
    mul = mybir.AluOpType.mult
    add = mybir.AluOpType.add
    eq = mybir.AluOpType.is_equal
    mx = mybir.AluOpType.max

    nc = bacc.Bacc(
        "TRN2",
        target_bir_lowering=False,
        debug=False,
        enable_asserts=False,
        num_devices=ncores,
        num_swdge_queues=cfg.get("nqueues", 4),
    )

    x_own = nc.dram_tensor("x_own", [nblk * 128, F], f32, kind="ExternalInput")
    W1d = nc.dram_tensor("W1", [F, F], f32, kind="ExternalInput")
    W2d = nc.dram_tensor("W2", [F, F], f32, kind="ExternalInput")
    b1d = nc.dram_tensor("b1_bc", [128, F], f32, kind="ExternalInput")
    b2d = nc.dram_tensor("b2_bc", [128, F], f32, kind="ExternalInput")
    degd = nc.dram_tensor("deg", [128, nblk], f32, kind="ExternalInput")
    iotad = nc.dram_tensor("iota", [128, 128], TD, kind="ExternalInput")
    ixlod = ixhid = dllod = dlhid = None
    if S_lo:
        ixlod = nc.dram_tensor("idx_lo", [128, S_lo // 16], i16, kind="ExternalInput")
        dllod = nc.dram_tensor("dl_lo", [128, S_lo // 128], TD, kind="ExternalInput")
    if S_hi:
        ixhid = nc.dram_tensor("idx_hi", [128, S_hi // 16], i16, kind="ExternalInput")
        dlhid = nc.dram_tensor("dl_hi", [128, S_hi // 128], TD, kind="ExternalInput")
    yout = nc.dram_tensor("y_out", [rows, F], f32, kind="ExternalOutput")

    h_stage = [nc.dram_tensor(f"h_stage{i}", [rows, F], TD) for i in (1, 2)]
    h_full = [
        nc.dram_tensor(f"h_full{i}", [N, F], TD, addr_space="Shared") for i in (1, 2)
    ]

    with tile.TileContext(nc) as tc, ExitStack() as ctx:
        const = ctx.enter_context(tc.tile_pool(name="const", bufs=1))
        xf = ctx.enter_context(tc.tile_pool(name="xf", bufs=3))
        xfp = ctx.enter_context(tc.tile_pool(name="xfp", bufs=2, space="PSUM"))
        gps = ctx.enter_context(tc.tile_pool(name="gps", bufs=2, space="PSUM"))
        glo = ctx.enter_context(tc.tile_pool(name="glo", bufs=2))
        ghi = ctx.enter_context(tc.tile_pool(name="ghi", bufs=2))
        selp = ctx.enter_context(tc.tile_pool(name="selp", bufs=4))
        outp = ctx.enter_context(tc.tile_pool(name="outp", bufs=3))

        # ---- constants ---------------------------------------------------
        def load_const(dram, shape, dtype):
            t = const.tile(shape, dtype, tag=f"c_{dram.name}")
            nc.sync.dma_start(t[:], dram[:])
            return t

        W1s = load_const(W1d, [F, F], f32)
        W2s = load_const(W2d, [F, F], f32)
        b1s = load_const(b1d, [128, F], f32)
        b2s = load_const(b2d, [128, F], f32)
        iota = load_const(iotad, [128, 128], TD)
        degs = load_const(degd, [128, nblk], f32)
        ixlo = load_const(ixlod, [128, S_lo // 16], i16) if S_lo else None
        dllo = load_const(dllod, [128, S_lo // 128], TD) if S_lo else None
        ixhi = load_const(ixhid, [128, S_hi // 16], i16) if S_hi else None
        dlhi = load_const(dlhid, [128, S_hi // 128], TD) if S_hi else None

        dinv = const.tile([128, nblk], f32)
        nc.vector.reciprocal(dinv[:], degs[:])
        nc.scalar.activation(dinv[:], dinv[:], mybir.ActivationFunctionType.Sqrt)

        ident = const.tile([128, 128], f32)
        make_identity(nc, ident[:])

        y1 = const.tile([128, nblk * 128], f32)  # layer-1 output, feature cols

        # ---- transform: h_stage = dinv * (rows @ W) ----------------------
        def transform(get_tile, W_s, stage):
            for t in range(nblk):
                xt = get_tile(t)
                pT = xfp.tile([128, 128], f32)
                nc.tensor.transpose(pT[:], xt[:], ident[:])
                xT = xf.tile([128, 128], f32)
                nc.vector.tensor_copy(xT[:], pT[:])
                ph = xfp.tile([128, F], f32)
                nc.tensor.matmul(ph[:], lhsT=xT[:], rhs=W_s[:], start=True, stop=True)
                hs = xf.tile([128, F], TD)
                nc.vector.tensor_scalar(
                    out=hs[:], in0=ph[:], scalar1=dinv[:, t : t + 1], scalar2=None,
                    op0=mul,
                )
                r = 128 if t < nblk - 1 else last_rows
                nc.sync.dma_start(stage[t * 128 : t * 128 + r, :], hs[:r, :])

        def x_tile(t):
            xt = xf.tile([128, F], f32)
            nc.sync.dma_start(xt[:], x_own[t * 128 : (t + 1) * 128, :])
            return xt

        # ---- gather + segment-sum reduce ---------------------------------
        qctr = [0]

        class Stream:
            def __init__(self, idx, dl, view, total_chunks, pool):
                self.idx, self.dl, self.view, self.pool = idx, dl, view, pool
                self.total = total_chunks
                self.pos = 0
                self.slab = None
                self.base = 0
                self.n = 0

            def chunk(self):
                """Return (slab_tile, column) for the chunk at self.pos."""
                if self.slab is None or self.pos >= self.base + self.n:
                    self.base = self.pos
                    self.n = min(gchunks, self.total - self.pos)
                    nidx = self.n * 128
                    self.slab = self.pool.tile([128, self.n, F], TD, tag="slab")
                    nc.gpsimd.dma_gather(
                        self.slab[:],
                        self.view,
                        self.idx[:, self.pos * 8 : self.pos * 8 + nidx // 16],
                        nidx,
                        nidx,
                        F,
                        queue_num=qctr[0] % nq,
                    )
                    qctr[0] += 1
                col = self.pos - self.base
                self.pos += 1
                return self.slab, col

        def gather_reduce(hf, finalize):
            streams = []
            if S_lo:
                streams.append(
                    (c_lo, Stream(ixlo, dllo, hf[:], S_lo // 128, glo))
                )
            if S_hi:
                streams.append(
                    (c_hi, Stream(ixhi, dlhi, hf[split:, :], S_hi // 128, ghi))
                )
            for b in range(nblk):
                py = gps.tile([128, F], f32)
                nch = sum(c[b] for c, _ in streams)
                assert nch > 0
                i = 0
                for c, st in streams:
                    done = 0
                    while done < c[b]:
                        g = min(selb, c[b] - done)
                        p0 = st.pos
                        sel = selp.tile([128, selb, 128], TD)
                        nc.vector.tensor_tensor(
                            out=sel[:, :g, :],
                            in0=st.dl[:, p0 : p0 + g].to_broadcast([128, g, 128]),
                            in1=iota[:, None, :].to_broadcast([128, g, 128]),
                            op=eq,
                        )
                        for j in range(g):
                            slab, col = st.chunk()
                            nc.tensor.matmul(
                                py[:],
                                lhsT=sel[:, j, :],
                                rhs=slab[:, col, :],
                                start=(i == 0),
                                stop=(i == nch - 1),
                            )
                            i += 1
                        done += g
                finalize(b, py)

        # ---- layer 1 -----------------------------------------------------
        transform(x_tile, W1s, h_stage[0])
        nc.gpsimd.collective_compute(
            "AllGather",
            mybir.AluOpType.bypass,
            replica_groups=[list(range(ncores))],
            ins=[h_stage[0][:]],
            outs=[h_full[0][:]],
        )

        def fin1(b, py):
            ys = y1[:, b * 128 : (b + 1) * 128]
            nc.vector.scalar_tensor_tensor(
                out=ys, in0=py[:], scalar=dinv[:, b : b + 1], in1=b1s[:],
                op0=mul, op1=add,
            )
            nc.vector.tensor_scalar(
                out=ys, in0=ys, scalar1=0.0, scalar2=None, op0=mx
            )

        gather_reduce(h_full[0], fin1)

        # ---- layer 2 -----------------------------------------------------
        transform(lambda t: y1[:, t * 128 : (t + 1) * 128], W2s, h_stage[1])
        nc.gpsimd.collective_compute(
            "AllGather",
            mybir.AluOpType.bypass,
            replica_groups=[list(range(ncores))],
            ins=[h_stage[1][:]],
            outs=[h_full[1][:]],
        )

        def fin2(b, py):
            yt = outp.tile([128, F], f32)
            nc.vector.scalar_tensor_tensor(
                out=yt[:], in0=py[:], scalar=dinv[:, b : b + 1], in1=b2s[:],
                op0=mul, op1=add,
            )
            r = 128 if b < nblk - 1 else last_rows
            nc.sync.dma_start(yout[b * 128 : b * 128 + r, :], yt[:r, :])

        gather_reduce(h_full[1], fin2)

    nc.compile()
    return nc


def get_program(cfg, sched):
    key = (tuple(sorted(cfg.items())), sched)
    if key not in _PROGRAM_CACHE:
        _PROGRAM_CACHE[key] = build_program(cfg, sched)
    return _PROGRAM_CACHE[key]


# ----------------------------------------------------------------------------
# input marshalling + entry point
# ----------------------------------------------------------------------------


def make_in_maps(x, W1, b1, W2, b2, cfg, per_core):
    N, F, ncores, rows, nblk = (
        cfg["N"],
        cfg["F"],
        cfg["ncores"],
        cfg["rows"],
        cfg["nblk"],
    )
    import ml_dtypes

    td = ml_dtypes.bfloat16 if cfg["table_bf16"] else np.float32
    x = np.asarray(x, np.float32)
    W1 = np.ascontiguousarray(np.asarray(W1, np.float32))
    W2 = np.ascontiguousarray(np.asarray(W2, np.float32))
    b1_bc = np.ascontiguousarray(np.broadcast_to(np.asarray(b1, np.float32), (128, F)))
    b2_bc = np.ascontiguousarray(np.broadcast_to(np.asarray(b2, np.float32), (128, F)))
    iota = np.ascontiguousarray(
        np.broadcast_to(np.arange(128, dtype=np.float32), (128, 128))
    ).astype(td)
    in_maps = []
    for k in range(ncores):
        xk = np.zeros((nblk * 128, F), np.float32)
        xk[:rows] = x[k * rows : (k + 1) * rows]
        pc = per_core[k]
        in_maps.append(
            dict(
                x_own=xk,
                W1=W1,
                W2=W2,
                b1_bc=b1_bc,
                b2_bc=b2_bc,
                deg=pc["deg"],
                iota=iota,
                idx_lo=pc["idx_lo"],
                idx_hi=pc["idx_hi"],
                dl_lo=pc["dl_lo"].astype(td),
                dl_hi=pc["dl_hi"].astype(td),
            )
        )
    return in_maps


def _ensure_ntff_hook():
    """Register the NTFF profiling hook (missing antenv.axon_hooks shim)."""
    try:
        from antenv.axon_hooks import get_axon_ntff_profile_hook  # noqa: F401

        return True
    except ImportError:
        pass
    try:
        import sys
        import types

        import antenv
        from trn_agent_boot.trn_boot import _ntff_profile_via_ctypes

        hook = _ntff_profile_via_ctypes("/opt/axon/libaxon_pjrt.so")
        if hook is None:
            return False
        mod = types.ModuleType("antenv.axon_hooks")
        mod._hook = hook
        mod.get_axon_ntff_profile_hook = lambda: mod._hook
        mod.set_axon_ntff_profile_hook = lambda h: setattr(mod, "_hook", h)
        sys.modules["antenv.axon_hooks"] = mod
        antenv.axon_hooks = mod
        # artifact upload needs cloud credentials; stub it out
        import concourse.bass_utils as bu

        bu.upload_artifacts = lambda tmpdir: f"local:{tmpdir}"
        return True
    except Exception:
        return False


def run(x, edge_index, W1, b1, W2, b2, cfg, trace=False):
    from concourse.bass_utils import run_bass_kernel_spmd

    if trace:
        trace = _ensure_ntff_hook()

    sched, per_core, _ = preprocess(edge_index, cfg)
    nc = get_program(cfg, sched)
    in_maps = make_in_maps(x, W1, b1, W2, b2, cfg, per_core)
    res = run_bass_kernel_spmd(
        nc, in_maps, list(range(cfg["ncores"])), trace=trace
    )
    out = np.concatenate(
        [res.results[k]["y_out"] for k in range(cfg["ncores"])], axis=0
    )
    return out.astype(np.float32), res


def kernel(x, edge_index, W1, b1, W2, b2):
    out, _ = run(x, edge_index, W1, b1, W2, b2, FULL_CFG)
    return out


# revision 17
# speedup vs baseline: 2.6898x; 1.6281x over previous
"""Two-layer GCN feature extractor on 8 Trainium2 NeuronCores.

Strategy (self-contained; all shapes hardcoded for the target problem):
  * Nodes are sharded across 8 cores (6250 rows each).  Each core:
      1. transforms its own rows: h = dinv * (x_own @ W)         (PE)
      2. AllGather -> replicated h table [N, F] in DRAM          (collective)
      3. gathers per-edge source rows with dma_gather            (SWDGE DMA)
      4. segment-sums messages into its destination rows via
         one-hot selection matmuls accumulated in PSUM           (PE + DVE)
      5. y = relu(dinv * psum + b)  (layer 1), then repeat for layer 2.
  * Graph preprocessing (edge partitioning by destination, sorting,
    degree counting, int16 index stream layout) happens on host with
    numpy; all floating-point math runs on device.
  * dma_gather needs int16 indices, so the node table is addressed as
    two halves: src < 32768 ("lo", table base row 0) and src >= 32768
    ("hi", table base row 32768).
"""

import math
import os
from contextlib import ExitStack

import numpy as np

os.environ.setdefault("MYCRO_LOCAL_CACHE", "1")

# ----------------------------------------------------------------------------
# configuration
# ----------------------------------------------------------------------------


def make_cfg(
    N=50000,
    F=128,
    ncores=8,
    split=32768,
    gchunks=8,
    table_bf16=False,
    nqueues=4,
    selb=32,
):
    assert N % ncores == 0
    rows = N // ncores
    nblk = math.ceil(rows / 128)
    return dict(
        N=N,
        F=F,
        ncores=ncores,
        split=split,
        rows=rows,
        nblk=nblk,
        last_rows=rows - (nblk - 1) * 128,
        gchunks=gchunks,
        table_bf16=table_bf16,
        nqueues=nqueues,
        selb=selb,
    )


FULL_CFG = make_cfg()


# ----------------------------------------------------------------------------
# host-side graph preprocessing
# ----------------------------------------------------------------------------


def preprocess(edge_index, cfg):
    """Partition edges by destination core, sort by (dst block, src half),
    pad each (block, half) run to a multiple of 128, and lay out index /
    dst-local streams in the formats dma_gather and the kernel expect.

    Returns (sched, per_core, deg) where sched = (c_lo, c_hi) chunk counts
    per block (uniform across cores).
    """
    N, ncores, rows, nblk, split = (
        cfg["N"],
        cfg["ncores"],
        cfg["rows"],
        cfg["nblk"],
        cfg["split"],
    )

    src = np.asarray(edge_index[0], dtype=np.int64)
    dst = np.asarray(edge_index[1], dtype=np.int64)
    loops = np.arange(N, dtype=np.int64)
    src = np.concatenate([src, loops])
    dst = np.concatenate([dst, loops])

    deg = np.bincount(dst, minlength=N).astype(np.float32)  # >= 1 (self loops)

    core_of = dst // rows
    per_core_raw = []
    counts = np.zeros((ncores, nblk, 2), dtype=np.int64)
    for k in range(ncores):
        m = core_of == k
        s_k = src[m]
        d_k = dst[m] - k * rows
        blk = d_k >> 7
        dl = (d_k & 127).astype(np.float32)
        half = (s_k >= split).astype(np.int64)
        order = np.lexsort((s_k, half, blk))
        s_k, dl, blk, half = s_k[order], dl[order], blk[order], half[order]
        c = np.bincount(blk * 2 + half, minlength=nblk * 2).reshape(nblk, 2)
        counts[k] = c
        per_core_raw.append((s_k, dl, c))

    cdiv = lambda a, b: -(-a // b)
    c_lo = [int(max(cdiv(counts[k, b, 0], 128) for k in range(ncores))) for b in range(nblk)]
    c_hi = [int(max(cdiv(counts[k, b, 1], 128) for k in range(ncores))) for b in range(nblk)]
    S_lo = sum(c_lo) * 128
    S_hi = sum(c_hi) * 128

    per_core = []
    for k in range(ncores):
        s_k, dl_k, c = per_core_raw[k]
        # per-(block, half) start offsets into the sorted arrays
        starts = np.concatenate([[0], np.cumsum(c.reshape(-1))])
        idx_lo = np.zeros(S_lo, np.int16)
        dst_lo = np.full(S_lo, -1.0, np.float32)
        idx_hi = np.zeros(S_hi, np.int16)
        dst_hi = np.full(S_hi, -1.0, np.float32)
        plo = phi = 0
        for b in range(nblk):
            n0 = int(c[b, 0])
            o0 = int(starts[b * 2])
            idx_lo[plo : plo + n0] = s_k[o0 : o0 + n0].astype(np.int16)
            dst_lo[plo : plo + n0] = dl_k[o0 : o0 + n0]
            plo += c_lo[b] * 128
            n1 = int(c[b, 1])
            o1 = int(starts[b * 2 + 1])
            idx_hi[phi : phi + n1] = (s_k[o1 : o1 + n1] - split).astype(np.int16)
            dst_hi[phi : phi + n1] = dl_k[o1 : o1 + n1]
            phi += c_hi[b] * 128

        def arrange_idx(a):  # logical i -> sbuf[i % 16, i // 16], tiled to 128 parts
            if a.size == 0:
                return np.zeros((128, 0), np.int16)
            return np.tile(np.ascontiguousarray(a.reshape(-1, 16).T), (8, 1))

        def arrange_dl(a):  # logical i -> sbuf[i % 128, i // 128]
            if a.size == 0:
                return np.zeros((128, 0), np.float32)
            return np.ascontiguousarray(a.reshape(-1, 128).T)

        degk = np.ones(nblk * 128, np.float32)
        degk[:rows] = deg[k * rows : (k + 1) * rows]

        per_core.append(
            dict(
                idx_lo=arrange_idx(idx_lo),
                idx_hi=arrange_idx(idx_hi),
                dl_lo=arrange_dl(dst_lo),
                dl_hi=arrange_dl(dst_hi),
                deg=np.ascontiguousarray(degk.reshape(nblk, 128).T),
            )
        )

    return (tuple(c_lo), tuple(c_hi)), per_core, deg


# ----------------------------------------------------------------------------
# bass program
# ----------------------------------------------------------------------------

_PROGRAM_CACHE = {}


def build_program(cfg, sched):
    import concourse.bacc as bacc
    import concourse.bass as bass
    import concourse.mybir as mybir
    import concourse.tile as tile
    from concourse.masks import make_identity

    c_lo, c_hi = sched
    N, F, ncores, split = cfg["N"], cfg["F"], cfg["ncores"], cfg["split"]
    rows, nblk, last_rows = cfg["rows"], cfg["nblk"], cfg["last_rows"]
    gchunks = cfg["gchunks"]
    S_lo, S_hi = sum(c_lo) * 128, sum(c_hi) * 128

    f32 = mybir.dt.float32
    i16 = mybir.dt.int16
    TD = mybir.dt.bfloat16 if cfg["table_bf16"] else f32
    nq = cfg.get("nqueues", 4)
    selb = cfg.get("selb", 4)
    mul = mybir.AluOpType.mult
    add = mybir.AluOpType.add
    eq = mybir.AluOpType.is_equal
    mx = mybir.AluOpType.max

    nc = bacc.Bacc(
        "TRN2",
        target_bir_lowering=False,
        debug=False,
        enable_asserts=False,
        num_devices=ncores,
        num_swdge_queues=cfg.get("nqueues", 4),
    )

    x_own = nc.dram_tensor("x_own", [nblk * 128, F], f32, kind="ExternalInput")
    W1d = nc.dram_tensor("W1", [F, F], f32, kind="ExternalInput")
    W2d = nc.dram_tensor("W2", [F, F], f32, kind="ExternalInput")
    b1d = nc.dram_tensor("b1_bc", [128, F], f32, kind="ExternalInput")
    b2d = nc.dram_tensor("b2_bc", [128, F], f32, kind="ExternalInput")
    degd = nc.dram_tensor("deg", [128, nblk], f32, kind="ExternalInput")
    iotad = nc.dram_tensor("iota", [128, 128], TD, kind="ExternalInput")
    ixlod = ixhid = dllod = dlhid = None
    if S_lo:
        ixlod = nc.dram_tensor("idx_lo", [128, S_lo // 16], i16, kind="ExternalInput")
        dllod = nc.dram_tensor("dl_lo", [128, S_lo // 128], TD, kind="ExternalInput")
    if S_hi:
        ixhid = nc.dram_tensor("idx_hi", [128, S_hi // 16], i16, kind="ExternalInput")
        dlhid = nc.dram_tensor("dl_hi", [128, S_hi // 128], TD, kind="ExternalInput")
    yout = nc.dram_tensor("y_out", [rows, F], f32, kind="ExternalOutput")

    h_stage = [nc.dram_tensor(f"h_stage{i}", [rows, F], TD) for i in (1, 2)]
    h_full = [
        nc.dram_tensor(f"h_full{i}", [N, F], TD, addr_space="Shared") for i in (1, 2)
    ]

    with tile.TileContext(nc) as tc, ExitStack() as ctx:
        const = ctx.enter_context(tc.tile_pool(name="const", bufs=1))
        xf = ctx.enter_context(tc.tile_pool(name="xf", bufs=3))
        xfp = ctx.enter_context(tc.tile_pool(name="xfp", bufs=2, space="PSUM"))
        gps = ctx.enter_context(tc.tile_pool(name="gps", bufs=2, space="PSUM"))
        glo = ctx.enter_context(tc.tile_pool(name="glo", bufs=5))
        ghi = ctx.enter_context(tc.tile_pool(name="ghi", bufs=5))
        selp = ctx.enter_context(tc.tile_pool(name="selp", bufs=3))
        outp = ctx.enter_context(tc.tile_pool(name="outp", bufs=3))

        # ---- constants ---------------------------------------------------
        def load_const(dram, shape, dtype):
            t = const.tile(shape, dtype, tag=f"c_{dram.name}")
            nc.sync.dma_start(t[:], dram[:])
            return t

        W1s = load_const(W1d, [F, F], f32)
        W2s = load_const(W2d, [F, F], f32)
        b1s = load_const(b1d, [128, F], f32)
        b2s = load_const(b2d, [128, F], f32)
        iota = load_const(iotad, [128, 128], TD)
        degs = load_const(degd, [128, nblk], f32)
        ixlo = load_const(ixlod, [128, S_lo // 16], i16) if S_lo else None
        dllo = load_const(dllod, [128, S_lo // 128], TD) if S_lo else None
        ixhi = load_const(ixhid, [128, S_hi // 16], i16) if S_hi else None
        dlhi = load_const(dlhid, [128, S_hi // 128], TD) if S_hi else None

        dinv = const.tile([128, nblk], f32)
        nc.vector.reciprocal(dinv[:], degs[:])
        nc.scalar.activation(dinv[:], dinv[:], mybir.ActivationFunctionType.Sqrt)

        ident = const.tile([128, 128], f32)
        make_identity(nc, ident[:])

        y1 = const.tile([128, nblk * 128], f32)  # layer-1 output, feature cols

        # ---- transform: h_stage = dinv * (rows @ W) ----------------------
        def transform(get_tile, W_s, stage):
            for t in range(nblk):
                xt = get_tile(t)
                pT = xfp.tile([128, 128], f32)
                nc.tensor.transpose(pT[:], xt[:], ident[:])
                xT = xf.tile([128, 128], f32)
                nc.vector.tensor_copy(xT[:], pT[:])
                ph = xfp.tile([128, F], f32)
                nc.tensor.matmul(ph[:], lhsT=xT[:], rhs=W_s[:], start=True, stop=True)
                hs = xf.tile([128, F], TD)
                nc.vector.tensor_scalar(
                    out=hs[:], in0=ph[:], scalar1=dinv[:, t : t + 1], scalar2=None,
                    op0=mul,
                )
                r = 128 if t < nblk - 1 else last_rows
                nc.sync.dma_start(stage[t * 128 : t * 128 + r, :], hs[:r, :])

        def x_tile(t):
            xt = xf.tile([128, F], f32)
            nc.sync.dma_start(xt[:], x_own[t * 128 : (t + 1) * 128, :])
            return xt

        # ---- gather + segment-sum reduce ---------------------------------
        qctr = [0]

        class Stream:
            def __init__(self, idx, dl, view, total_chunks, pool):
                self.idx, self.dl, self.view, self.pool = idx, dl, view, pool
                self.total = total_chunks
                self.pos = 0
                self.slab = None
                self.base = 0
                self.n = 0

            def chunk(self):
                """Return (slab_tile, column) for the chunk at self.pos."""
                if self.slab is None or self.pos >= self.base + self.n:
                    self.base = self.pos
                    self.n = min(gchunks, self.total - self.pos)
                    nidx = self.n * 128
                    self.slab = self.pool.tile([128, self.n, F], TD, tag="slab")
                    nc.gpsimd.dma_gather(
                        self.slab[:],
                        self.view,
                        self.idx[:, self.pos * 8 : self.pos * 8 + nidx // 16],
                        nidx,
                        nidx,
                        F,
                        queue_num=qctr[0] % nq,
                    )
                    qctr[0] += 1
                col = self.pos - self.base
                self.pos += 1
                return self.slab, col

        def gather_reduce(hf, finalize):
            streams = []
            if S_lo:
                streams.append(
                    (c_lo, Stream(ixlo, dllo, hf[:], S_lo // 128, glo))
                )
            if S_hi:
                streams.append(
                    (c_hi, Stream(ixhi, dlhi, hf[split:, :], S_hi // 128, ghi))
                )
            for b in range(nblk):
                py = gps.tile([128, F], f32)
                nch = sum(c[b] for c, _ in streams)
                assert nch > 0
                i = 0
                for c, st in streams:
                    done = 0
                    while done < c[b]:
                        g = min(selb, c[b] - done)
                        p0 = st.pos
                        sel = selp.tile([128, selb, 128], TD)
                        nc.vector.tensor_tensor(
                            out=sel[:, :g, :],
                            in0=st.dl[:, p0 : p0 + g].to_broadcast([128, g, 128]),
                            in1=iota[:, None, :].to_broadcast([128, g, 128]),
                            op=eq,
                        )
                        for j in range(g):
                            slab, col = st.chunk()
                            nc.tensor.matmul(
                                py[:],
                                lhsT=sel[:, j, :],
                                rhs=slab[:, col, :],
                                start=(i == 0),
                                stop=(i == nch - 1),
                            )
                            i += 1
                        done += g
                finalize(b, py)

        # ---- layer 1 -----------------------------------------------------
        transform(x_tile, W1s, h_stage[0])
        nc.gpsimd.collective_compute(
            "AllGather",
            mybir.AluOpType.bypass,
            replica_groups=[list(range(ncores))],
            ins=[h_stage[0][:]],
            outs=[h_full[0][:]],
        )

        def fin1(b, py):
            ys = y1[:, b * 128 : (b + 1) * 128]
            nc.vector.scalar_tensor_tensor(
                out=ys, in0=py[:], scalar=dinv[:, b : b + 1], in1=b1s[:],
                op0=mul, op1=add,
            )
            nc.vector.tensor_scalar(
                out=ys, in0=ys, scalar1=0.0, scalar2=None, op0=mx
            )

        gather_reduce(h_full[0], fin1)

        # ---- layer 2 -----------------------------------------------------
        transform(lambda t: y1[:, t * 128 : (t + 1) * 128], W2s, h_stage[1])
        nc.gpsimd.collective_compute(
            "AllGather",
            mybir.AluOpType.bypass,
            replica_groups=[list(range(ncores))],
            ins=[h_stage[1][:]],
            outs=[h_full[1][:]],
        )

        def fin2(b, py):
            yt = outp.tile([128, F], f32)
            nc.vector.scalar_tensor_tensor(
                out=yt[:], in0=py[:], scalar=dinv[:, b : b + 1], in1=b2s[:],
                op0=mul, op1=add,
            )
            r = 128 if b < nblk - 1 else last_rows
            nc.sync.dma_start(yout[b * 128 : b * 128 + r, :], yt[:r, :])

        gather_reduce(h_full[1], fin2)

    nc.compile()
    return nc


def get_program(cfg, sched):
    key = (tuple(sorted(cfg.items())), sched)
    if key not in _PROGRAM_CACHE:
        _PROGRAM_CACHE[key] = build_program(cfg, sched)
    return _PROGRAM_CACHE[key]


# ----------------------------------------------------------------------------
# input marshalling + entry point
# ----------------------------------------------------------------------------


def make_in_maps(x, W1, b1, W2, b2, cfg, per_core):
    N, F, ncores, rows, nblk = (
        cfg["N"],
        cfg["F"],
        cfg["ncores"],
        cfg["rows"],
        cfg["nblk"],
    )
    import ml_dtypes

    td = ml_dtypes.bfloat16 if cfg["table_bf16"] else np.float32
    x = np.asarray(x, np.float32)
    W1 = np.ascontiguousarray(np.asarray(W1, np.float32))
    W2 = np.ascontiguousarray(np.asarray(W2, np.float32))
    b1_bc = np.ascontiguousarray(np.broadcast_to(np.asarray(b1, np.float32), (128, F)))
    b2_bc = np.ascontiguousarray(np.broadcast_to(np.asarray(b2, np.float32), (128, F)))
    iota = np.ascontiguousarray(
        np.broadcast_to(np.arange(128, dtype=np.float32), (128, 128))
    ).astype(td)
    in_maps = []
    for k in range(ncores):
        xk = np.zeros((nblk * 128, F), np.float32)
        xk[:rows] = x[k * rows : (k + 1) * rows]
        pc = per_core[k]
        in_maps.append(
            dict(
                x_own=xk,
                W1=W1,
                W2=W2,
                b1_bc=b1_bc,
                b2_bc=b2_bc,
                deg=pc["deg"],
                iota=iota,
                idx_lo=pc["idx_lo"],
                idx_hi=pc["idx_hi"],
                dl_lo=pc["dl_lo"].astype(td),
                dl_hi=pc["dl_hi"].astype(td),
            )
        )
    return in_maps


def _ensure_ntff_hook():
    """Register the NTFF profiling hook (missing antenv.axon_hooks shim)."""
    try:
        from antenv.axon_hooks import get_axon_ntff_profile_hook  # noqa: F401

        return True
    except ImportError:
        pass
    try:
        import sys
        import types

        import antenv
        from trn_agent_boot.trn_boot import _ntff_profile_via_ctypes

        hook = _ntff_profile_via_ctypes("/opt/axon/libaxon_pjrt.so")
        if hook is None:
            return False
        mod = types.ModuleType("antenv.axon_hooks")
        mod._hook = hook
        mod.get_axon_ntff_profile_hook = lambda: mod._hook
        mod.set_axon_ntff_profile_hook = lambda h: setattr(mod, "_hook", h)
        sys.modules["antenv.axon_hooks"] = mod
        antenv.axon_hooks = mod
        # artifact upload needs cloud credentials; stub it out
        import concourse.bass_utils as bu

        bu.upload_artifacts = lambda tmpdir: f"local:{tmpdir}"
        return True
    except Exception:
        return False


def run(x, edge_index, W1, b1, W2, b2, cfg, trace=False):
    from concourse.bass_utils import run_bass_kernel_spmd

    if trace:
        trace = _ensure_ntff_hook()

    sched, per_core, _ = preprocess(edge_index, cfg)
    nc = get_program(cfg, sched)
    in_maps = make_in_maps(x, W1, b1, W2, b2, cfg, per_core)
    res = run_bass_kernel_spmd(
        nc, in_maps, list(range(cfg["ncores"])), trace=trace
    )
    out = np.concatenate(
        [res.results[k]["y_out"] for k in range(cfg["ncores"])], axis=0
    )
    return out.astype(np.float32), res


def kernel(x, edge_index, W1, b1, W2, b2):
    out, _ = run(x, edge_index, W1, b1, W2, b2, FULL_CFG)
    return out
